# revision 10
# baseline (speedup 1.0000x reference)
"""DeBERTa-style 12-layer transformer forward on 8 Trainium2 NeuronCores.

Strategy: data-parallel over batch (B=8 -> 1 sequence per core). Each core runs
the full 12-layer model on its [512, 768] activation, kept TRANSPOSED in SBUF
([HID on partitions, S on free]) so every projection is a plain PE matmul with
naturally-laid-out weights. No collectives.

DeBERTa relative-position terms: since S == M == 512, rel(q,k) = q-k+512 never
clips.  With host-reversed positional projections (pk_rev[r'] = pk[1023-r']):
    c2p[q, k]  = qpk_rev[q, 511-q+k]   (same-partition diagonal slice)
    p2cT[k, q] = kpq_rev[k, 511-k+q]   (same-partition diagonal slice)
Both are per-partition shifted contiguous row reads, which a SBUF->SBUF DMA
expresses as a single 3-dim access pattern with partition stride (pitch-1).
c2p is then block-transposed on the PE into [k, q] orientation, and p2cT is
accumulated onto it with an accumulating SWDGE DMA.

Softmax: logits are small (inputs are LN'd, weights ~0.02), so exp without
max-subtraction is numerically safe. The key-padding mask multiplies V rows
and the appended ones-column of V~ = [V*m | m], whose matmul row 64 yields the
softmax denominator for free; division is applied to the [64, 512] per-head
context via a TensorE reciprocal broadcast.

LayerNorm gain/bias are folded into the following projection weights on the
host; attention scale 1/sqrt(3D) is folded into Wq and Wpq.
"""

import math
import os
import sys

import numpy as np

sys.path.insert(0, "/opt/trn_rl_repo")

import ml_dtypes  # noqa: F401

import concourse.bass as bass
import concourse.mybir as mybir
import concourse.tile as tile
from concourse import bacc
from concourse.bass_utils import run_bass_kernel_spmd
from concourse.masks import make_identity

dt = mybir.dt
AF = mybir.ActivationFunctionType
ALU = mybir.AluOpType

B, S, HID, L, NH, V, M = 8, 512, 768, 12, 12, 30522, 512
FF = 4 * HID
D = HID // NH           # 64
CT = HID // 128         # 6 c-tiles
ST = S // 128           # 4 s-tiles
FT = FF // 128          # 24 f-tiles
FTH = FT // 2           # 12 (FFN half)
R2 = 2 * M              # 1024
RT = R2 // 512          # 2 chunks of the r axis
BAND = 640              # needed qpk band width per 128-row tile
SCALE = 1.0 / math.sqrt(3.0 * D)
EPS = 1e-5

N_LAYERS = int(os.environ.get("KERNEL_N_LAYERS", str(L)))
ABLATE = set(x for x in os.environ.get("KERNEL_ABLATE", "").split(",") if x)
N_CORES = 8

bf16 = dt.bfloat16
f32 = dt.float32
f32r = dt.float32r


def _diag_ap(t, n_tiles, band, width, base):
    """AP reading t[p, i, base - p + k] for k in [0, width): per-partition
    backward-shifted contiguous rows, one DMA for all n_tiles sub-tiles."""
    src = t[:, :, :].copy()
    pitch = src.ap[0][0]
    out = src.copy()
    out.ap = src.ap[:0] + [[pitch - 1, 128], [band, n_tiles], [1, width]]
    out.offset = src.offset + base
    return out


def build(n_layers=N_LAYERS):
    nc = bacc.Bacc(None, target_bir_lowering=False, debug=False)
    with tile.TileContext(nc) as tc:
        with tc.tile_pool(name="dram", bufs=1, space="DRAM") as dram:
            tok_d = dram.tile([S, 1], dt.int32, kind="ExternalInput", name="token_ids", uniquify=False)
            seg_d = dram.tile([S, 1], dt.int32, kind="ExternalInput", name="segment_info", uniquify=False)
            emb_d = dram.tile([V, HID], f32, kind="ExternalInput", name="tok_emb", uniquify=False)
            sege_d = dram.tile([3, HID], f32, kind="ExternalInput", name="seg_emb", uniquify=False)
            pe_d = dram.tile([128, ST, HID], f32, kind="ExternalInput", name="pe", uniquify=False)
            lnfg_d = dram.tile([128, CT], f32, kind="ExternalInput", name="lnf_g", uniquify=False)
            lnfb_d = dram.tile([128, CT], f32, kind="ExternalInput", name="lnf_b", uniquify=False)
            out_d = dram.tile([128, CT, S], f32, kind="ExternalOutput", name="out", uniquify=False)

            WD = {}
            for l in range(n_layers):
                for nm, sh in [("wq", [128, CT, HID]), ("wk", [128, CT, HID]),
                               ("wv", [128, CT, HID]), ("wo", [128, CT, HID]),
                               ("pk", [128, CT, R2]), ("pq", [128, CT, R2]),
                               ("w1a", [128, CT, FF // 2]), ("w1b", [128, CT, FF // 2]),
                               ("w2a", [128, FTH, HID]), ("w2b", [128, FTH, HID])]:
                    WD[nm, l] = dram.tile(sh, bf16, kind="ExternalInput", name=f"{nm}_{l}", uniquify=False)
                for nm, sh in [("bq", [128, CT]), ("bk", [128, CT]), ("bo", [128, CT]),
                               ("b1", [128, FT]), ("b2", [128, CT])]:
                    WD[nm, l] = dram.tile(sh, f32, kind="ExternalInput", name=f"{nm}_{l}", uniquify=False)

            with nc.allow_low_precision("bf16/f32r kernel by design"):
                _build_body(nc, tc, n_layers, tok_d, seg_d, emb_d, sege_d, pe_d,
                            lnfg_d, lnfb_d, out_d, WD)

    nc.compile()
    return nc


def _build_body(nc, tc, n_layers, tok_d, seg_d, emb_d, sege_d, pe_d,
                lnfg_d, lnfb_d, out_d, WD):
    with tc.tile_pool(name="persist", bufs=1) as pers, \
         tc.tile_pool(name="whid", bufs=2) as whid, \
         tc.tile_pool(name="wff", bufs=2) as wff, \
         tc.tile_pool(name="wb", bufs=2) as wb, \
         tc.tile_pool(name="act", bufs=1) as act, \
         tc.tile_pool(name="scr", bufs=1) as scr, \
         tc.tile_pool(name="hscr", bufs=2) as hscr, \
         tc.tile_pool(name="bnd", bufs=2) as bnd, \
         tc.tile_pool(name="hscr2", bufs=2) as hscr2, \
         tc.tile_pool(name="ppA", bufs=4, space="PSUM") as ppA, \
         tc.tile_pool(name="ppB", bufs=2, space="PSUM") as ppB:

        # ---------------- constants ----------------
        ident_f = pers.tile([128, 128], f32, name="ident_f")
        make_identity(nc, ident_f[:, :])
        ident_b = pers.tile([128, 128], bf16, name="ident_b")
        nc.vector.tensor_copy(out=ident_b[:, :], in_=ident_f[:, :])
        ones_f = pers.tile([128, 1], f32, name="ones_f")
        nc.vector.memset(ones_f[:, :], 1.0)
        ones_col_r = pers.tile([128, 1], f32r, name="ones_col_r")
        nc.vector.tensor_copy(out=ones_col_r[:, :], in_=ones_f[:, :])
        ones_rf = pers.tile([1, 128], f32, name="ones_rf")
        nc.vector.memset(ones_rf[:, :], 1.0)
        ones_row_r = pers.tile([1, 128], f32r, name="ones_row_r")
        nc.vector.tensor_copy(out=ones_row_r[:, :], in_=ones_rf[:, :])
        eps_t = pers.tile([1, 1], f32, name="eps_t")
        nc.vector.memset(eps_t[:, :], EPS)

        # ---------------- embedding (DMA-only accumulate) ----------------
        tok_sb = pers.tile([128, ST], dt.int32, name="tok_sb")
        seg_sb = pers.tile([128, ST], dt.int32, name="seg_sb")
        for t in range(ST):
            nc.sync.dma_start(out=tok_sb[:, t:t + 1], in_=tok_d[128 * t:128 * (t + 1), :])
            nc.sync.dma_start(out=seg_sb[:, t:t + 1], in_=seg_d[128 * t:128 * (t + 1), :])

        m01_f = pers.tile([128, ST], f32, name="m01_f")
        m01_b = pers.tile([128, ST], bf16, name="m01_b")
        nc.vector.tensor_scalar(out=m01_f[:, :], in0=tok_sb[:, :], scalar1=0,
                                scalar2=None, op0=ALU.not_equal)
        nc.vector.tensor_copy(out=m01_b[:, :], in_=m01_f[:, :])

        xtmp = act.tile([128, ST, HID], f32, name="xtmp")
        x_nat = xtmp
        nc.sync.dma_start(out=x_nat[:, :, :], in_=pe_d[:, :, :])
        for t in range(ST):
            nc.gpsimd.indirect_dma_start(
                out=x_nat[:, t, :], out_offset=None, in_=emb_d[:, :],
                in_offset=bass.IndirectOffsetOnAxis(ap=tok_sb[:, t:t + 1], axis=0),
                compute_op=ALU.add)
            nc.gpsimd.indirect_dma_start(
                out=x_nat[:, t, :], out_offset=None, in_=sege_d[:, :],
                in_offset=bass.IndirectOffsetOnAxis(ap=seg_sb[:, t:t + 1], axis=0),
                compute_op=ALU.add)

        # transpose x_nat [S, HID] -> xT [HID, S] (f32r residual)
        xT = act.tile([128, CT, S], f32r, name="xT")
        for ct in range(CT):
            tp = ppA.tile([128, 512], f32, name="ps", tag="ps")
            for t in range(ST):
                nc.tensor.transpose(out=tp[:, 128 * t:128 * (t + 1)],
                                    in_=x_nat[:, t, 128 * ct:128 * (ct + 1)],
                                    identity=ident_f[:, :])
            nc.vector.tensor_copy(out=xT[:, ct, :], in_=tp[:, :])

        # ---------------- helper: layernorm ----------------
        def layernorm(x_in, h_out, g_ap=None, b_ap=None):
            if not callable(x_in):
                x_t = x_in; x_in = lambda kt: x_t[:, kt, :]
            if not callable(h_out):
                h_t = h_out; h_out = lambda ct: h_t[:, ct, :]
            sum_ps = ppA.tile([1, 512], f32, name="ln_sum", tag="ps")
            for kt in range(CT):
                nc.tensor.matmul(sum_ps[:, :], ones_col_r[:, :], x_in(kt),
                                 start=(kt == 0), stop=(kt == CT - 1))
            sq_ps = ppA.tile([1, 512], f32, name="ln_sq", tag="ps")
            for kt in range(CT):
                x2 = scr.tile([128, 512], f32r, name="ln_x2")
                nc.scalar.activation(out=x2[:, :], in_=x_in(kt), func=AF.Square)
                nc.tensor.matmul(sq_ps[:, :], ones_col_r[:, :], x2[:, :],
                                 start=(kt == 0), stop=(kt == CT - 1))
            mu = scr.tile([1, 512], f32, name="ln_mu")
            nc.vector.tensor_scalar(out=mu[:, :], in0=sum_ps[:, :], scalar1=1.0 / HID,
                                    scalar2=None, op0=ALU.mult)
            msq = scr.tile([1, 512], f32, name="ln_msq")
            nc.vector.tensor_tensor(out=msq[:, :], in0=mu[:, :], in1=mu[:, :], op=ALU.mult)
            var = scr.tile([1, 512], f32, name="ln_var")
            nc.vector.scalar_tensor_tensor(out=var[:, :], in0=sq_ps[:, :], scalar=1.0 / HID,
                                           in1=msq[:, :], op0=ALU.mult, op1=ALU.subtract)
            sig = scr.tile([1, 512], f32, name="ln_sig")
            nc.scalar.activation(out=sig[:, :], in_=var[:, :], func=AF.Sqrt, bias=eps_t[:, :])
            stack = scr.tile([1, R2], f32r, name="ln_stack")
            nc.vector.reciprocal(out=stack[:, 0:512], in_=sig[:, :])
            nc.vector.tensor_tensor(out=stack[:, 512:R2], in0=mu[:, :],
                                    in1=stack[:, 0:512], op=ALU.mult)
            maps = scr.tile([128, R2], f32r, name="ln_maps")
            for c in range(2):
                mp = ppA.tile([128, 512], f32, name="ln_mp", tag="ps")
                nc.tensor.matmul(mp[:, :], ones_row_r[:1, :], stack[:, 512 * c:512 * (c + 1)],
                                 start=True, stop=True)
                nc.scalar.activation(out=maps[:, 512 * c:512 * (c + 1)], in_=mp[:, :], func=AF.Copy)
            for ct in range(CT):
                t1 = scr.tile([128, 512], f32, name="ln_t1")
                nc.vector.tensor_tensor(out=t1[:, :], in0=x_in(ct),
                                        in1=maps[:, 0:512], op=ALU.mult)
                if g_ap is None:
                    nc.vector.tensor_tensor(out=h_out(ct), in0=t1[:, :],
                                            in1=maps[:, 512:R2], op=ALU.subtract)
                else:
                    t2 = scr.tile([128, 512], f32, name="ln_t2")
                    nc.vector.tensor_tensor(out=t2[:, :], in0=t1[:, :],
                                            in1=maps[:, 512:R2], op=ALU.subtract)
                    nc.scalar.activation(out=h_out(ct), in_=t2[:, :], func=AF.Identity,
                                         scale=g_ap[:, ct:ct + 1], bias=b_ap[:, ct:ct + 1])

        hT = act.tile([128, CT, S], bf16, name="hT")
        qT = act.tile([128, CT, S], bf16, name="qT")
        kT = act.tile([128, CT, S], bf16, name="kT")
        vS = act.tile([128, ST, NH, D + 1], bf16, name="vS")
        pkT = act.tile([128, CT, R2], bf16, name="pkT")
        pqT = act.tile([128, CT, R2], bf16, name="pqT")
        ctxT = act.tile([128, CT, S], bf16, name="ctxT")
        m1q = act.tile([128, CT, S], bf16, name="m1q")

        # ---------------- layers ----------------
        reps = int(os.environ.get("KERNEL_REPS", "1"))
        for l in [x % n_layers for x in range(n_layers * reps)]:
            # ---- LN1 -> hT (bf16)
            layernorm(xT, hT)

            # ---- q/k projections (transposed out) with folded bias
            for wi, (wnm, bnm, o_t) in enumerate([("wq", "bq", qT), ("wk", "bk", kT)]):
                w_t = whid.tile([128, CT, HID], bf16, name=wnm, tag="whid")
                eng = nc.sync if wi == 0 else nc.scalar
                eng.dma_start(out=w_t[:, :, :], in_=WD[wnm, l][:, :, :])
                b_t = wb.tile([128, CT], f32, name=bnm, tag="wbias")
                nc.sync.dma_start(out=b_t[:, :], in_=WD[bnm, l][:, :])
                for ot in range(CT):
                    pp = ppA.tile([128, 512], f32, name="proj_ps", tag="ps")
                    for kt in range(CT):
                        nc.tensor.matmul(pp[:, :], w_t[:, kt, 128 * ot:128 * (ot + 1)],
                                         hT[:, kt, :], start=(kt == 0), stop=(kt == CT - 1))
                    nc.scalar.activation(out=o_t[:, ot, :], in_=pp[:, :], func=AF.Identity,
                                         bias=b_t[:, ot:ot + 1])

            # ---- v natural [s, h, d] with mask folded; ones column = m01
            wv = whid.tile([128, CT, HID], bf16, name="wv", tag="whid")
            nc.scalar.dma_start(out=wv[:, :, :], in_=WD["wv", l][:, :, :])
            for st in range(ST):
                for c in range(2):  # two 384-wide chunks of d
                    pp = ppA.tile([128, 512], f32, name="v_ps", tag="ps")
                    for kt in range(CT):
                        nc.tensor.matmul(pp[:, 0:384], hT[:, kt, 128 * st:128 * (st + 1)],
                                         wv[:, kt, 384 * c:384 * (c + 1)],
                                         start=(kt == 0), stop=(kt == CT - 1))
                    src = pp[:, 0:384].rearrange("p (h d) -> p h d", d=D)
                    nc.scalar.activation(out=vS[:, st, 6 * c:6 * (c + 1), 0:D], in_=src,
                                         func=AF.Copy, scale=m01_f[:, st:st + 1])
                icol = m01_b[:, st:st + 1].copy()
                icol.ap = icol.ap[:0] + [[icol.ap[0][0], 128], [0, NH], [1, 1]]
                nc.vector.tensor_copy(out=vS[:, st, :, D:D + 1], in_=icol)

            # ---- positional projections precomputed on host; just load
            nc.sync.dma_start(out=pkT[:, :, :], in_=WD["pk", l][:, :, :])
            nc.scalar.dma_start(out=pqT[:, :, :], in_=WD["pq", l][:, :, :])

            # ---- attention per head
            for h in range(NH if "attn" not in ABLATE else 0):
                hp = 64 * (h % 2)
                hc = h // 2
                q_h = qT[hp:hp + 64, hc, :]
                k_h = kT[hp:hp + 64, hc, :]

                b2_t = bnd.tile([128, 2 * ST, BAND], bf16, name="band2")
                for si, (src_T, posT) in enumerate([(qT, pkT), (kT, pqT)] if "bands" not in ABLATE else []):
                    s_h = src_T[hp:hp + 64, hc, :]
                    p_h = posT[hp:hp + 64, hc, :]
                    for t in range(ST):
                        bs = 384 - 128 * t
                        bp = ppB.tile([128, BAND], f32, name="band_ps", tag="band")
                        nc.tensor.matmul(bp[:, 0:512], s_h[:, 128 * t:128 * (t + 1)],
                                         p_h[:, bs:bs + 512], start=True, stop=True)
                        nc.tensor.matmul(bp[:, 512:BAND], s_h[:, 128 * t:128 * (t + 1)],
                                         p_h[:, bs + 512:bs + BAND], start=True, stop=True)
                        if t % 2 == 0:
                            nc.scalar.activation(out=b2_t[:, ST * si + t, :], in_=bp[:, :], func=AF.Copy)
                        else:
                            nc.vector.tensor_copy(out=b2_t[:, ST * si + t, :], in_=bp[:, :])

                # one diagonal skew DMA for both c2p [q,k] (subs 0..3) and
                # p2cT [k,q] (subs 4..7)
                sk2 = hscr.tile([128, 2 * ST, 512], bf16, name="sk2")
                if "skew" not in ABLATE:
                    eng = nc.sync if h % 2 == 0 else nc.scalar
                    eng.dma_start(out=sk2[:, :, :], in_=_diag_ap(b2_t, 2 * ST, BAND, 512, 127))
                c2p = sk2[:, 0:ST, :]
                p2cT = sk2[:, ST:2 * ST, :]
                # pos[k,q] = transpose(c2p) + p2cT, fused into the PSUM eviction
                pos = hscr.tile([128, ST, 512], bf16, name="pos")
                for u in range(ST):
                    tp = ppA.tile([128, 512], bf16, name="c2pT_ps", tag="ps")
                    if "transpose" not in ABLATE:
                        for t in range(ST):
                            nc.tensor.transpose(out=tp[:, 128 * t:128 * (t + 1)],
                                                in_=c2p[:, t, 128 * u:128 * (u + 1)],
                                                identity=ident_b[:, :])
                    if "posadd" not in ABLATE:
                        nc.vector.tensor_tensor(out=pos[:, u, :], in0=tp[:, :],
                                                in1=p2cT[:, u, :], op=ALU.add)

                # scoresT per k-tile: c2c (PSUM) + pos -> exp (bf16) -> AV
                av = ppA.tile([128, 512], f32, name="av_ps", tag="ps")
                for u in range(ST):
                    cp = ppA.tile([128, 512], f32, name="c2c_ps", tag="ps")
                    nc.tensor.matmul(cp[:, :], k_h[:, 128 * u:128 * (u + 1)], q_h[:, :],
                                     start=True, stop=True)
                    sf = hscr2.tile([128, 512], f32, name="s_f32")
                    ex = hscr2.tile([128, 512], bf16, name="expT")
                    if "exp" not in ABLATE:
                        nc.vector.tensor_tensor(out=sf[:, :], in0=cp[:, :], in1=pos[:, u, :], op=ALU.add)
                        nc.scalar.activation(out=ex[:, :], in_=sf[:, :], func=AF.Exp)
                    nc.tensor.matmul(av[0:D + 1, :], vS[:, u, h, :], ex[:, :],
                                     start=(u == 0), stop=(u == ST - 1))

                rec = hscr2.tile([1, 512], f32r, name="rec")
                nc.vector.reciprocal(out=rec[:, :], in_=av[D:D + 1, :])
                rm = ppA.tile([128, 512], f32, name="rmap_ps", tag="ps")
                nc.tensor.matmul(rm[0:D, :], ones_row_r[:1, 0:D], rec[:, :], start=True, stop=True)
                rms = hscr2.tile([64, 512], f32r, name="rmap_sb")
                nc.scalar.activation(out=rms[:, :], in_=rm[0:D, :], func=AF.Copy)
                nc.vector.tensor_tensor(out=ctxT[hp:hp + 64, hc, :], in0=av[0:D, :],
                                        in1=rms[:, :], op=ALU.mult)

            # ---- attention out proj + residual
            wo = whid.tile([128, CT, HID], bf16, name="wo", tag="whid")
            nc.sync.dma_start(out=wo[:, :, :], in_=WD["wo", l][:, :, :])
            bo = wb.tile([128, CT], f32, name="bo", tag="wbias")
            nc.sync.dma_start(out=bo[:, :], in_=WD["bo", l][:, :])
            for ot in range(CT):
                pp = ppA.tile([128, 512], f32, name="wo_ps", tag="ps")
                for kt in range(CT):
                    nc.tensor.matmul(pp[:, :], wo[:, kt, 128 * ot:128 * (ot + 1)],
                                     ctxT[:, kt, :], start=(kt == 0), stop=(kt == CT - 1))
                nc.vector.scalar_tensor_tensor(out=xT[:, ot, :], in0=pp[:, :],
                                               scalar=bo[:, ot:ot + 1], in1=xT[:, ot, :],
                                               op0=ALU.add, op1=ALU.add)

            # ---- LN2 -> h2 (reuse hT)
            layernorm(xT, hT)

            # ---- FFN in quarters (m1 buffer reused; xT updated per quarter)
            b1 = wb.tile([128, FT], f32, name="b1", tag="wb1")
            nc.sync.dma_start(out=b1[:, :], in_=WD["b1", l][:, :])
            b2 = wb.tile([128, CT], f32, name="b2", tag="wbias")
            nc.sync.dma_start(out=b2[:, :], in_=WD["b2", l][:, :])
            for qtr in range(4 if "ffn" not in ABLATE else 0):
                half, qh = divmod(qtr, 2)
                w1nm = "w1a" if half == 0 else "w1b"
                w2nm = "w2a" if half == 0 else "w2b"
                w1 = wff.tile([128, CT, 768], bf16, name="w1", tag="wff")
                nc.scalar.dma_start(out=w1[:, :, :], in_=WD[w1nm, l][:, :, 768 * qh:768 * (qh + 1)])
                for ft in range(CT):
                    pp = ppA.tile([128, 512], f32, name="ff1_ps", tag="ps")
                    for kt in range(CT):
                        nc.tensor.matmul(pp[:, :], w1[:, kt, 128 * ft:128 * (ft + 1)],
                                         hT[:, kt, :], start=(kt == 0), stop=(kt == CT - 1))
                    fg = CT * qtr + ft
                    nc.scalar.activation(out=m1q[:, ft, :], in_=pp[:, :], func=AF.Gelu,
                                         bias=b1[:, fg:fg + 1])
                w2 = wff.tile([128, CT, HID], bf16, name="w2", tag="wff")
                nc.sync.dma_start(out=w2[:, :, :], in_=WD[w2nm, l][:, 6 * qh:6 * (qh + 1), :])
                for ot in range(CT):
                    pp = ppA.tile([128, 512], f32, name="ff2_ps", tag="ps")
                    for kt in range(CT):
                        nc.tensor.matmul(pp[:, :], w2[:, kt, 128 * ot:128 * (ot + 1)],
                                         m1q[:, kt, :], start=(kt == 0), stop=(kt == CT - 1))
                    if qtr < 3:
                        nc.vector.tensor_tensor(out=xT[:, ot, :], in0=pp[:, :],
                                                in1=xT[:, ot, :], op=ALU.add)
                    else:
                        nc.vector.scalar_tensor_tensor(out=xT[:, ot, :], in0=pp[:, :],
                                                       scalar=b2[:, ot:ot + 1], in1=xT[:, ot, :],
                                                       op0=ALU.add, op1=ALU.add)

        # ---------------- final LN + output ----------------
        lnfg = pers.tile([128, CT], f32, name="lnfg")
        lnfb = pers.tile([128, CT], f32, name="lnfb")
        nc.sync.dma_start(out=lnfg[:, :], in_=lnfg_d[:, :])
        nc.sync.dma_start(out=lnfb[:, :], in_=lnfb_d[:, :])
        xf_base = xtmp[:, :, :].copy()
        pitch = xf_base.ap[0][0]
        def xf_view(ct):
            v = xf_base.copy()
            v.ap = xf_base.ap[:0] + [[pitch, 128], [1, 512]]
            v.offset = xf_base.offset + 512 * ct
            return v
        layernorm(xT, xf_view, g_ap=lnfg, b_ap=lnfb)
        xf_all = xf_base.copy()
        xf_all.ap = xf_base.ap[:0] + [[pitch, 128], [512, CT], [1, 512]]
        nc.sync.dma_start(out=out_d[:, :, :], in_=xf_all)


# ---------------------------------------------------------------------------
# host side
# ---------------------------------------------------------------------------

def _tile_kxm(w):
    """[K, N] -> [128, K//128, N] partition-tiled."""
    K, N = w.shape
    return np.ascontiguousarray(w.reshape(K // 128, 128, N).transpose(1, 0, 2))


def _tile_bias(b):
    n = b.shape[0]
    return np.ascontiguousarray(b.reshape(n // 128, 128).T)


def _sinusoidal_pe(seq_len, dim):
    pos = np.arange(seq_len, dtype=np.float32)[:, None]
    i = np.arange(dim // 2, dtype=np.float32)[None, :]
    ang = pos / np.power(10000.0, 2.0 * i / dim)
    pe = np.zeros((seq_len, dim), np.float32)
    pe[:, 0::2] = np.sin(ang)
    pe[:, 1::2] = np.cos(ang)
    return pe


def prep_weights(inputs, n_layers=N_LAYERS):
    f = lambda k: np.asarray(inputs[k], np.float32)
    wmap = {}
    wmap["tok_emb"] = f("tok_emb")
    wmap["seg_emb"] = f("seg_emb")
    pe = _sinusoidal_pe(S, HID)
    wmap["pe"] = np.ascontiguousarray(pe.reshape(ST, 128, HID).transpose(1, 0, 2))
    wmap["lnf_g"] = _tile_bias(f("lnf_g"))
    wmap["lnf_b"] = _tile_bias(f("lnf_b"))
    bfl = lambda x: _tile_kxm(x).astype(ml_dtypes.bfloat16)
    for l in range(n_layers):
        g1, b1g = f("ln1_g")[l], f("ln1_b")[l]
        g2, b2g = f("ln2_g")[l], f("ln2_b")[l]
        Wq, Wk, Wv, Wo = f("Wq")[l], f("Wk")[l], f("Wv")[l], f("Wo")[l]
        Wpk, Wpq = f("Wpk")[l], f("Wpq")[l]
        rel = f("rel_emb")[l]
        W1, W2 = f("W1")[l], f("W2")[l]
        bq, bk, bv, bo = f("bq")[l], f("bk")[l], f("bv")[l], f("bo")[l]
        b1, b2 = f("b1")[l], f("b2")[l]

        pk_rev = (rel[::-1] @ Wpk)            # [R2, HID] reversed r
        pq_rev = (rel[::-1] @ (Wpq * SCALE))
        wq_ = (g1[:, None] * Wq) * SCALE
        bq_ = (b1g @ Wq + bq) * SCALE
        wk_ = g1[:, None] * Wk
        bk_ = b1g @ Wk + bk
        bv_ = b1g @ Wv + bv
        bo_ = bv_ @ Wo + bo
        w1_ = g2[:, None] * W1
        b1_ = b2g @ W1 + b1
        wmap[f"wq_{l}"] = bfl(wq_)
        wmap[f"wk_{l}"] = bfl(wk_)
        wmap[f"wv_{l}"] = bfl(g1[:, None] * Wv)
        wmap[f"wo_{l}"] = bfl(Wo)
        wmap[f"pk_{l}"] = bfl(np.ascontiguousarray(pk_rev.T))
        wmap[f"pq_{l}"] = bfl(np.ascontiguousarray(pq_rev.T))
        w1t = bfl(w1_)
        wmap[f"w1a_{l}"] = np.ascontiguousarray(w1t[:, :, :FF // 2])
        wmap[f"w1b_{l}"] = np.ascontiguousarray(w1t[:, :, FF // 2:])
        w2t = bfl(W2)
        wmap[f"w2a_{l}"] = np.ascontiguousarray(w2t[:, :FTH, :])
        wmap[f"w2b_{l}"] = np.ascontiguousarray(w2t[:, FTH:, :])
        wmap[f"bq_{l}"] = _tile_bias(bq_)
        wmap[f"bk_{l}"] = _tile_bias(bk_)
        wmap[f"bo_{l}"] = _tile_bias(bo_)
        wmap[f"b1_{l}"] = _tile_bias(b1_)
        wmap[f"b2_{l}"] = _tile_bias(b2)
    return wmap


_NC_CACHE = {}


def _get_nc(n_layers=N_LAYERS):
    if n_layers not in _NC_CACHE:
        _NC_CACHE[n_layers] = build(n_layers)
    return _NC_CACHE[n_layers]


def run(inputs, n_layers=N_LAYERS, trace=False):
    nc = _get_nc(n_layers)
    wmap = prep_weights(inputs, n_layers)
    tok = np.asarray(inputs["token_ids"], np.int32)
    seg = np.asarray(inputs["segment_info"], np.int32)
    in_maps = []
    for b in range(B):
        m = dict(wmap)
        m["token_ids"] = np.ascontiguousarray(tok[b].reshape(S, 1))
        m["segment_info"] = np.ascontiguousarray(seg[b].reshape(S, 1))
        in_maps.append(m)
    res = run_bass_kernel_spmd(nc, in_maps, core_ids=list(range(N_CORES)), trace=trace)
    outs = []
    for b in range(B):
        o = res.results[b]["out"]                      # [128, CT, S]
        o = o.transpose(1, 0, 2).reshape(HID, S).T     # -> [S, HID]
        outs.append(o)
    out = np.stack(outs).astype(np.float32)
    return out, res


def kernel(**inputs):
    out, _ = run(inputs)
    return out


# revision 11
# speedup vs baseline: 1.0637x; 1.0637x over previous
"""DeBERTa-style 12-layer transformer forward on 8 Trainium2 NeuronCores.

Strategy: data-parallel over batch (B=8 -> 1 sequence per core). Each core runs
the full 12-layer model on its [512, 768] activation, kept TRANSPOSED in SBUF
([HID on partitions, S on free]) so every projection is a plain PE matmul with
naturally-laid-out weights. No collectives.

DeBERTa relative-position terms: since S == M == 512, rel(q,k) = q-k+512 never
clips.  With host-reversed positional projections (pk_rev[r'] = pk[1023-r']):
    c2p[q, k]  = qpk_rev[q, 511-q+k]   (same-partition diagonal slice)
    p2cT[k, q] = kpq_rev[k, 511-k+q]   (same-partition diagonal slice)
Both are per-partition shifted contiguous row reads, which a SBUF->SBUF DMA
expresses as a single 3-dim access pattern with partition stride (pitch-1).
c2p is then block-transposed on the PE into [k, q] orientation, and p2cT is
accumulated onto it with an accumulating SWDGE DMA.

Softmax: logits are small (inputs are LN'd, weights ~0.02), so exp without
max-subtraction is numerically safe. The key-padding mask multiplies V rows
and the appended ones-column of V~ = [V*m | m], whose matmul row 64 yields the
softmax denominator for free; division is applied to the [64, 512] per-head
context via a TensorE reciprocal broadcast.

LayerNorm gain/bias are folded into the following projection weights on the
host; attention scale 1/sqrt(3D) is folded into Wq and Wpq.
"""

import math
import os
import sys

import numpy as np

sys.path.insert(0, "/opt/trn_rl_repo")

import ml_dtypes  # noqa: F401

import concourse.bass as bass
import concourse.mybir as mybir
import concourse.tile as tile
from concourse import bacc
from concourse.bass_utils import run_bass_kernel_spmd
from concourse.masks import make_identity

dt = mybir.dt
AF = mybir.ActivationFunctionType
ALU = mybir.AluOpType

B, S, HID, L, NH, V, M = 8, 512, 768, 12, 12, 30522, 512
FF = 4 * HID
D = HID // NH           # 64
CT = HID // 128         # 6 c-tiles
ST = S // 128           # 4 s-tiles
FT = FF // 128          # 24 f-tiles
FTH = FT // 2           # 12 (FFN half)
R2 = 2 * M              # 1024
RT = R2 // 512          # 2 chunks of the r axis
BAND = 640              # needed qpk band width per 128-row tile
SCALE = 1.0 / math.sqrt(3.0 * D)
EPS = 1e-5

N_LAYERS = int(os.environ.get("KERNEL_N_LAYERS", str(L)))
ABLATE = set(x for x in os.environ.get("KERNEL_ABLATE", "").split(",") if x)
N_CORES = 8

bf16 = dt.bfloat16
f32 = dt.float32
f32r = dt.float32r


def _diag_ap(t, n_tiles, band, width, base):
    """AP reading t[p, i, base - p + k] for k in [0, width): per-partition
    backward-shifted contiguous rows, one DMA for all n_tiles sub-tiles."""
    src = t[:, :, :].copy()
    pitch = src.ap[0][0]
    out = src.copy()
    out.ap = src.ap[:0] + [[pitch - 1, 128], [band, n_tiles], [1, width]]
    out.offset = src.offset + base
    return out


def build(n_layers=N_LAYERS):
    nc = bacc.Bacc(None, target_bir_lowering=False, debug=False)
    with tile.TileContext(nc) as tc:
        with tc.tile_pool(name="dram", bufs=1, space="DRAM") as dram:
            tok_d = dram.tile([S, 1], dt.int32, kind="ExternalInput", name="token_ids", uniquify=False)
            seg_d = dram.tile([S, 1], dt.int32, kind="ExternalInput", name="segment_info", uniquify=False)
            emb_d = dram.tile([V, HID], f32, kind="ExternalInput", name="tok_emb", uniquify=False)
            sege_d = dram.tile([3, HID], f32, kind="ExternalInput", name="seg_emb", uniquify=False)
            pe_d = dram.tile([128, ST, HID], f32, kind="ExternalInput", name="pe", uniquify=False)
            lnfg_d = dram.tile([128, CT], f32, kind="ExternalInput", name="lnf_g", uniquify=False)
            lnfb_d = dram.tile([128, CT], f32, kind="ExternalInput", name="lnf_b", uniquify=False)
            out_d = dram.tile([128, CT, S], f32, kind="ExternalOutput", name="out", uniquify=False)

            WD = {}
            for l in range(n_layers):
                for nm, sh in [("wq", [128, CT, HID]), ("wk", [128, CT, HID]),
                               ("wv", [128, CT, HID]), ("wo", [128, CT, HID]),
                               ("pk", [128, CT, R2]), ("pq", [128, CT, R2]),
                               ("w1a", [128, CT, FF // 2]), ("w1b", [128, CT, FF // 2]),
                               ("w2a", [128, FTH, HID]), ("w2b", [128, FTH, HID])]:
                    WD[nm, l] = dram.tile(sh, bf16, kind="ExternalInput", name=f"{nm}_{l}", uniquify=False)
                for nm, sh in [("bq", [128, CT]), ("bk", [128, CT]), ("bo", [128, CT]),
                               ("b1", [128, FT]), ("b2", [128, CT])]:
                    WD[nm, l] = dram.tile(sh, f32, kind="ExternalInput", name=f"{nm}_{l}", uniquify=False)

            with nc.allow_low_precision("bf16/f32r kernel by design"):
                _build_body(nc, tc, n_layers, tok_d, seg_d, emb_d, sege_d, pe_d,
                            lnfg_d, lnfb_d, out_d, WD)

    nc.compile()
    return nc


def _build_body(nc, tc, n_layers, tok_d, seg_d, emb_d, sege_d, pe_d,
                lnfg_d, lnfb_d, out_d, WD):
    with tc.tile_pool(name="persist", bufs=1) as pers, \
         tc.tile_pool(name="whid", bufs=2) as whid, \
         tc.tile_pool(name="wff", bufs=2) as wff, \
         tc.tile_pool(name="wb", bufs=2) as wb, \
         tc.tile_pool(name="act", bufs=1) as act, \
         tc.tile_pool(name="scr", bufs=1) as scr, \
         tc.tile_pool(name="hscr", bufs=2) as hscr, \
         tc.tile_pool(name="bnd", bufs=2) as bnd, \
         tc.tile_pool(name="hscr2", bufs=2) as hscr2, \
         tc.tile_pool(name="ppA", bufs=4, space="PSUM") as ppA, \
         tc.tile_pool(name="ppB", bufs=2, space="PSUM") as ppB:

        # ---------------- constants ----------------
        ident_f = pers.tile([128, 128], f32, name="ident_f")
        make_identity(nc, ident_f[:, :])
        ident_b = pers.tile([128, 128], bf16, name="ident_b")
        nc.vector.tensor_copy(out=ident_b[:, :], in_=ident_f[:, :])
        ones_f = pers.tile([128, 1], f32, name="ones_f")
        nc.vector.memset(ones_f[:, :], 1.0)
        ones_col_r = pers.tile([128, 1], f32r, name="ones_col_r")
        nc.vector.tensor_copy(out=ones_col_r[:, :], in_=ones_f[:, :])
        ones_rf = pers.tile([1, 128], f32, name="ones_rf")
        nc.vector.memset(ones_rf[:, :], 1.0)
        ones_row_r = pers.tile([1, 128], f32r, name="ones_row_r")
        nc.vector.tensor_copy(out=ones_row_r[:, :], in_=ones_rf[:, :])
        eps_t = pers.tile([1, 1], f32, name="eps_t")
        nc.vector.memset(eps_t[:, :], EPS)

        # ---------------- embedding (DMA-only accumulate) ----------------
        tok_sb = pers.tile([128, ST], dt.int32, name="tok_sb")
        seg_sb = pers.tile([128, ST], dt.int32, name="seg_sb")
        for t in range(ST):
            nc.sync.dma_start(out=tok_sb[:, t:t + 1], in_=tok_d[128 * t:128 * (t + 1), :])
            nc.sync.dma_start(out=seg_sb[:, t:t + 1], in_=seg_d[128 * t:128 * (t + 1), :])

        m01_f = pers.tile([128, ST], f32, name="m01_f")
        m01_b = pers.tile([128, ST], bf16, name="m01_b")
        nc.vector.tensor_scalar(out=m01_f[:, :], in0=tok_sb[:, :], scalar1=0,
                                scalar2=None, op0=ALU.not_equal)
        nc.vector.tensor_copy(out=m01_b[:, :], in_=m01_f[:, :])

        xtmp = act.tile([128, ST, HID], f32, name="xtmp")
        x_nat = xtmp
        nc.sync.dma_start(out=x_nat[:, :, :], in_=pe_d[:, :, :])
        for t in range(ST):
            nc.gpsimd.indirect_dma_start(
                out=x_nat[:, t, :], out_offset=None, in_=emb_d[:, :],
                in_offset=bass.IndirectOffsetOnAxis(ap=tok_sb[:, t:t + 1], axis=0),
                compute_op=ALU.add)
            nc.gpsimd.indirect_dma_start(
                out=x_nat[:, t, :], out_offset=None, in_=sege_d[:, :],
                in_offset=bass.IndirectOffsetOnAxis(ap=seg_sb[:, t:t + 1], axis=0),
                compute_op=ALU.add)

        # transpose x_nat [S, HID] -> xT [HID, S] (f32r residual)
        xT = act.tile([128, CT, S], f32r, name="xT")
        for ct in range(CT):
            tp = ppA.tile([128, 512], f32, name="ps", tag="ps")
            for t in range(ST):
                nc.tensor.transpose(out=tp[:, 128 * t:128 * (t + 1)],
                                    in_=x_nat[:, t, 128 * ct:128 * (ct + 1)],
                                    identity=ident_f[:, :])
            nc.vector.tensor_copy(out=xT[:, ct, :], in_=tp[:, :])

        # ---------------- helper: layernorm ----------------
        def layernorm(x_in, h_out, g_ap=None, b_ap=None):
            if not callable(x_in):
                x_t = x_in; x_in = lambda kt: x_t[:, kt, :]
            if not callable(h_out):
                h_t = h_out; h_out = lambda ct: h_t[:, ct, :]
            sum_ps = ppA.tile([1, 512], f32, name="ln_sum", tag="ps")
            for kt in range(CT):
                nc.tensor.matmul(sum_ps[:, :], ones_col_r[:, :], x_in(kt),
                                 start=(kt == 0), stop=(kt == CT - 1))
            sq_ps = ppA.tile([1, 512], f32, name="ln_sq", tag="ps")
            for kt in range(CT):
                x2 = scr.tile([128, 512], f32r, name="ln_x2")
                nc.scalar.activation(out=x2[:, :], in_=x_in(kt), func=AF.Square)
                nc.tensor.matmul(sq_ps[:, :], ones_col_r[:, :], x2[:, :],
                                 start=(kt == 0), stop=(kt == CT - 1))
            mu = scr.tile([1, 512], f32, name="ln_mu")
            nc.vector.tensor_scalar(out=mu[:, :], in0=sum_ps[:, :], scalar1=1.0 / HID,
                                    scalar2=None, op0=ALU.mult)
            msq = scr.tile([1, 512], f32, name="ln_msq")
            nc.vector.tensor_tensor(out=msq[:, :], in0=mu[:, :], in1=mu[:, :], op=ALU.mult)
            var = scr.tile([1, 512], f32, name="ln_var")
            nc.vector.scalar_tensor_tensor(out=var[:, :], in0=sq_ps[:, :], scalar=1.0 / HID,
                                           in1=msq[:, :], op0=ALU.mult, op1=ALU.subtract)
            sig = scr.tile([1, 512], f32, name="ln_sig")
            nc.scalar.activation(out=sig[:, :], in_=var[:, :], func=AF.Sqrt, bias=eps_t[:, :])
            stack = scr.tile([1, R2], f32r, name="ln_stack")
            nc.vector.reciprocal(out=stack[:, 0:512], in_=sig[:, :])
            nc.vector.tensor_tensor(out=stack[:, 512:R2], in0=mu[:, :],
                                    in1=stack[:, 0:512], op=ALU.mult)
            maps = scr.tile([128, R2], f32r, name="ln_maps")
            for c in range(2):
                mp = ppA.tile([128, 512], f32, name="ln_mp", tag="ps")
                nc.tensor.matmul(mp[:, :], ones_row_r[:1, :], stack[:, 512 * c:512 * (c + 1)],
                                 start=True, stop=True)
                nc.scalar.activation(out=maps[:, 512 * c:512 * (c + 1)], in_=mp[:, :], func=AF.Copy)
            for ct in range(CT):
                t1 = scr.tile([128, 512], f32, name="ln_t1")
                nc.vector.tensor_tensor(out=t1[:, :], in0=x_in(ct),
                                        in1=maps[:, 0:512], op=ALU.mult)
                if g_ap is None:
                    nc.vector.tensor_tensor(out=h_out(ct), in0=t1[:, :],
                                            in1=maps[:, 512:R2], op=ALU.subtract)
                else:
                    t2 = scr.tile([128, 512], f32, name="ln_t2")
                    nc.vector.tensor_tensor(out=t2[:, :], in0=t1[:, :],
                                            in1=maps[:, 512:R2], op=ALU.subtract)
                    nc.scalar.activation(out=h_out(ct), in_=t2[:, :], func=AF.Identity,
                                         scale=g_ap[:, ct:ct + 1], bias=b_ap[:, ct:ct + 1])

        hT = act.tile([128, CT, S], bf16, name="hT")
        qT = act.tile([128, CT, S], bf16, name="qT")
        kT = act.tile([128, CT, S], bf16, name="kT")
        vS = act.tile([128, ST, NH, D + 1], bf16, name="vS")
        pkT = act.tile([128, CT, R2], bf16, name="pkT")
        pqT = act.tile([128, CT, R2], bf16, name="pqT")
        ctxT = act.tile([128, CT, S], bf16, name="ctxT")
        m1q = act.tile([128, CT, S], bf16, name="m1q")

        # ---------------- layers ----------------
        reps = int(os.environ.get("KERNEL_REPS", "1"))
        for l in [x % n_layers for x in range(n_layers * reps)]:
            # ---- LN1 -> hT (bf16)
            layernorm(xT, hT)

            # ---- q/k projections (transposed out) with folded bias
            for wi, (wnm, bnm, o_t) in enumerate([("wq", "bq", qT), ("wk", "bk", kT)]):
                w_t = whid.tile([128, CT, HID], bf16, name=wnm, tag="whid")
                eng = nc.sync if wi == 0 else nc.scalar
                eng.dma_start(out=w_t[:, :, :], in_=WD[wnm, l][:, :, :])
                b_t = wb.tile([128, CT], f32, name=bnm, tag="wbias")
                nc.sync.dma_start(out=b_t[:, :], in_=WD[bnm, l][:, :])
                for ot in range(CT):
                    pp = ppA.tile([128, 512], f32, name="proj_ps", tag="ps")
                    for kt in range(CT):
                        nc.tensor.matmul(pp[:, :], w_t[:, kt, 128 * ot:128 * (ot + 1)],
                                         hT[:, kt, :], start=(kt == 0), stop=(kt == CT - 1))
                    nc.scalar.activation(out=o_t[:, ot, :], in_=pp[:, :], func=AF.Identity,
                                         bias=b_t[:, ot:ot + 1])

            # ---- v natural [s, h, d] with mask folded; ones column = m01
            wv = whid.tile([128, CT, HID], bf16, name="wv", tag="whid")
            nc.scalar.dma_start(out=wv[:, :, :], in_=WD["wv", l][:, :, :])
            for st in range(ST):
                for c in range(2):  # two 384-wide chunks of d
                    pp = ppA.tile([128, 512], f32, name="v_ps", tag="ps")
                    for kt in range(CT):
                        nc.tensor.matmul(pp[:, 0:384], hT[:, kt, 128 * st:128 * (st + 1)],
                                         wv[:, kt, 384 * c:384 * (c + 1)],
                                         start=(kt == 0), stop=(kt == CT - 1))
                    src = pp[:, 0:384].rearrange("p (h d) -> p h d", d=D)
                    nc.scalar.activation(out=vS[:, st, 6 * c:6 * (c + 1), 0:D], in_=src,
                                         func=AF.Copy, scale=m01_f[:, st:st + 1])
                icol = m01_b[:, st:st + 1].copy()
                icol.ap = icol.ap[:0] + [[icol.ap[0][0], 128], [0, NH], [1, 1]]
                nc.vector.tensor_copy(out=vS[:, st, :, D:D + 1], in_=icol)

            # ---- positional projections precomputed on host; just load
            nc.sync.dma_start(out=pkT[:, :, :], in_=WD["pk", l][:, :, :])
            nc.scalar.dma_start(out=pqT[:, :, :], in_=WD["pq", l][:, :, :])

            # ---- attention per head
            for h in range(NH if "attn" not in ABLATE else 0):
                hp = 64 * (h % 2)
                hc = h // 2
                q_h = qT[hp:hp + 64, hc, :]
                k_h = kT[hp:hp + 64, hc, :]

                bq_t = bnd.tile([128, ST, BAND], bf16, name="band_q")
                bk_t = bnd.tile([128, ST, BAND], bf16, name="band_k")
                for src_T, posT, bt in ([(qT, pkT, bq_t), (kT, pqT, bk_t)] if "bands" not in ABLATE else []):
                    s_h = src_T[hp:hp + 64, hc, :]
                    p_h = posT[hp:hp + 64, hc, :]
                    for t in range(ST):
                        bs = 384 - 128 * t
                        bp = ppB.tile([128, BAND], f32, name="band_ps", tag="band")
                        nc.tensor.matmul(bp[:, 0:512], s_h[:, 128 * t:128 * (t + 1)],
                                         p_h[:, bs:bs + 512], start=True, stop=True)
                        nc.tensor.matmul(bp[:, 512:BAND], s_h[:, 128 * t:128 * (t + 1)],
                                         p_h[:, bs + 512:bs + BAND], start=True, stop=True)
                        if t % 2 == 0:
                            nc.scalar.activation(out=bt[:, t, :], in_=bp[:, :], func=AF.Copy)
                        else:
                            nc.vector.tensor_copy(out=bt[:, t, :], in_=bp[:, :])

                # c2p [q, k] and p2cT [k, q] via diagonal skew DMAs, one per
                # HWDGE ring so they run concurrently
                c2p = hscr.tile([128, ST, 512], bf16, name="c2p")
                p2cT = hscr.tile([128, ST, 512], bf16, name="p2cT")
                if "skew" not in ABLATE:
                    e1, e2 = (nc.sync, nc.scalar) if h % 2 == 0 else (nc.scalar, nc.sync)
                    e1.dma_start(out=c2p[:, :, :], in_=_diag_ap(bq_t, ST, BAND, 512, 127))
                    e2.dma_start(out=p2cT[:, :, :], in_=_diag_ap(bk_t, ST, BAND, 512, 127))
                # pos[k,q] = transpose(c2p) + p2cT, fused into the PSUM eviction
                pos = hscr.tile([128, ST, 512], bf16, name="pos")
                for u in range(ST):
                    tp = ppA.tile([128, 512], bf16, name="c2pT_ps", tag="ps")
                    if "transpose" not in ABLATE:
                        for t in range(ST):
                            nc.tensor.transpose(out=tp[:, 128 * t:128 * (t + 1)],
                                                in_=c2p[:, t, 128 * u:128 * (u + 1)],
                                                identity=ident_b[:, :])
                    if "posadd" not in ABLATE:
                        nc.vector.tensor_tensor(out=pos[:, u, :], in0=tp[:, :],
                                                in1=p2cT[:, u, :], op=ALU.add)

                # scoresT per k-tile: c2c (PSUM) + pos -> exp (bf16) -> AV
                av = ppA.tile([128, 512], f32, name="av_ps", tag="ps")
                for u in range(ST):
                    cp = ppA.tile([128, 512], f32, name="c2c_ps", tag="ps")
                    nc.tensor.matmul(cp[:, :], k_h[:, 128 * u:128 * (u + 1)], q_h[:, :],
                                     start=True, stop=True)
                    sf = hscr2.tile([128, 512], f32, name="s_f32")
                    ex = hscr2.tile([128, 512], bf16, name="expT")
                    if "exp" not in ABLATE:
                        nc.vector.tensor_tensor(out=sf[:, :], in0=cp[:, :], in1=pos[:, u, :], op=ALU.add)
                        nc.scalar.activation(out=ex[:, :], in_=sf[:, :], func=AF.Exp)
                    nc.tensor.matmul(av[0:D + 1, :], vS[:, u, h, :], ex[:, :],
                                     start=(u == 0), stop=(u == ST - 1))

                rec = hscr2.tile([1, 512], f32r, name="rec")
                nc.vector.reciprocal(out=rec[:, :], in_=av[D:D + 1, :])
                rm = ppA.tile([128, 512], f32, name="rmap_ps", tag="ps")
                nc.tensor.matmul(rm[0:D, :], ones_row_r[:1, 0:D], rec[:, :], start=True, stop=True)
                rms = hscr2.tile([64, 512], f32r, name="rmap_sb")
                nc.scalar.activation(out=rms[:, :], in_=rm[0:D, :], func=AF.Copy)
                nc.vector.tensor_tensor(out=ctxT[hp:hp + 64, hc, :], in0=av[0:D, :],
                                        in1=rms[:, :], op=ALU.mult)

            # ---- attention out proj + residual
            wo = whid.tile([128, CT, HID], bf16, name="wo", tag="whid")
            nc.sync.dma_start(out=wo[:, :, :], in_=WD["wo", l][:, :, :])
            bo = wb.tile([128, CT], f32, name="bo", tag="wbias")
            nc.sync.dma_start(out=bo[:, :], in_=WD["bo", l][:, :])
            for ot in range(CT):
                pp = ppA.tile([128, 512], f32, name="wo_ps", tag="ps")
                for kt in range(CT):
                    nc.tensor.matmul(pp[:, :], wo[:, kt, 128 * ot:128 * (ot + 1)],
                                     ctxT[:, kt, :], start=(kt == 0), stop=(kt == CT - 1))
                nc.vector.scalar_tensor_tensor(out=xT[:, ot, :], in0=pp[:, :],
                                               scalar=bo[:, ot:ot + 1], in1=xT[:, ot, :],
                                               op0=ALU.add, op1=ALU.add)

            # ---- LN2 -> h2 (reuse hT)
            layernorm(xT, hT)

            # ---- FFN in quarters (m1 buffer reused; xT updated per quarter)
            b1 = wb.tile([128, FT], f32, name="b1", tag="wb1")
            nc.sync.dma_start(out=b1[:, :], in_=WD["b1", l][:, :])
            b2 = wb.tile([128, CT], f32, name="b2", tag="wbias")
            nc.sync.dma_start(out=b2[:, :], in_=WD["b2", l][:, :])
            for qtr in range(4 if "ffn" not in ABLATE else 0):
                half, qh = divmod(qtr, 2)
                w1nm = "w1a" if half == 0 else "w1b"
                w2nm = "w2a" if half == 0 else "w2b"
                w1 = wff.tile([128, CT, 768], bf16, name="w1", tag="wff")
                nc.scalar.dma_start(out=w1[:, :, :], in_=WD[w1nm, l][:, :, 768 * qh:768 * (qh + 1)])
                for ft in range(CT):
                    pp = ppA.tile([128, 512], f32, name="ff1_ps", tag="ps")
                    for kt in range(CT):
                        nc.tensor.matmul(pp[:, :], w1[:, kt, 128 * ft:128 * (ft + 1)],
                                         hT[:, kt, :], start=(kt == 0), stop=(kt == CT - 1))
                    fg = CT * qtr + ft
                    nc.scalar.activation(out=m1q[:, ft, :], in_=pp[:, :], func=AF.Gelu,
                                         bias=b1[:, fg:fg + 1])
                w2 = wff.tile([128, CT, HID], bf16, name="w2", tag="wff")
                nc.sync.dma_start(out=w2[:, :, :], in_=WD[w2nm, l][:, 6 * qh:6 * (qh + 1), :])
                for ot in range(CT):
                    pp = ppA.tile([128, 512], f32, name="ff2_ps", tag="ps")
                    for kt in range(CT):
                        nc.tensor.matmul(pp[:, :], w2[:, kt, 128 * ot:128 * (ot + 1)],
                                         m1q[:, kt, :], start=(kt == 0), stop=(kt == CT - 1))
                    if qtr < 3:
                        nc.vector.tensor_tensor(out=xT[:, ot, :], in0=pp[:, :],
                                                in1=xT[:, ot, :], op=ALU.add)
                    else:
                        nc.vector.scalar_tensor_tensor(out=xT[:, ot, :], in0=pp[:, :],
                                                       scalar=b2[:, ot:ot + 1], in1=xT[:, ot, :],
                                                       op0=ALU.add, op1=ALU.add)

        # ---------------- final LN + output ----------------
        lnfg = pers.tile([128, CT], f32, name="lnfg")
        lnfb = pers.tile([128, CT], f32, name="lnfb")
        nc.sync.dma_start(out=lnfg[:, :], in_=lnfg_d[:, :])
        nc.sync.dma_start(out=lnfb[:, :], in_=lnfb_d[:, :])
        xf_base = xtmp[:, :, :].copy()
        pitch = xf_base.ap[0][0]
        def xf_view(ct):
            v = xf_base.copy()
            v.ap = xf_base.ap[:0] + [[pitch, 128], [1, 512]]
            v.offset = xf_base.offset + 512 * ct
            return v
        layernorm(xT, xf_view, g_ap=lnfg, b_ap=lnfb)
        xf_all = xf_base.copy()
        xf_all.ap = xf_base.ap[:0] + [[pitch, 128], [512, CT], [1, 512]]
        nc.sync.dma_start(out=out_d[:, :, :], in_=xf_all)


# ---------------------------------------------------------------------------
# host side
# ---------------------------------------------------------------------------

def _tile_kxm(w):
    """[K, N] -> [128, K//128, N] partition-tiled."""
    K, N = w.shape
    return np.ascontiguousarray(w.reshape(K // 128, 128, N).transpose(1, 0, 2))


def _tile_bias(b):
    n = b.shape[0]
    return np.ascontiguousarray(b.reshape(n // 128, 128).T)


def _sinusoidal_pe(seq_len, dim):
    pos = np.arange(seq_len, dtype=np.float32)[:, None]
    i = np.arange(dim // 2, dtype=np.float32)[None, :]
    ang = pos / np.power(10000.0, 2.0 * i / dim)
    pe = np.zeros((seq_len, dim), np.float32)
    pe[:, 0::2] = np.sin(ang)
    pe[:, 1::2] = np.cos(ang)
    return pe


def prep_weights(inputs, n_layers=N_LAYERS):
    f = lambda k: np.asarray(inputs[k], np.float32)
    wmap = {}
    wmap["tok_emb"] = f("tok_emb")
    wmap["seg_emb"] = f("seg_emb")
    pe = _sinusoidal_pe(S, HID)
    wmap["pe"] = np.ascontiguousarray(pe.reshape(ST, 128, HID).transpose(1, 0, 2))
    wmap["lnf_g"] = _tile_bias(f("lnf_g"))
    wmap["lnf_b"] = _tile_bias(f("lnf_b"))
    bfl = lambda x: _tile_kxm(x).astype(ml_dtypes.bfloat16)
    for l in range(n_layers):
        g1, b1g = f("ln1_g")[l], f("ln1_b")[l]
        g2, b2g = f("ln2_g")[l], f("ln2_b")[l]
        Wq, Wk, Wv, Wo = f("Wq")[l], f("Wk")[l], f("Wv")[l], f("Wo")[l]
        Wpk, Wpq = f("Wpk")[l], f("Wpq")[l]
        rel = f("rel_emb")[l]
        W1, W2 = f("W1")[l], f("W2")[l]
        bq, bk, bv, bo = f("bq")[l], f("bk")[l], f("bv")[l], f("bo")[l]
        b1, b2 = f("b1")[l], f("b2")[l]

        pk_rev = (rel[::-1] @ Wpk)            # [R2, HID] reversed r
        pq_rev = (rel[::-1] @ (Wpq * SCALE))
        wq_ = (g1[:, None] * Wq) * SCALE
        bq_ = (b1g @ Wq + bq) * SCALE
        wk_ = g1[:, None] * Wk
        bk_ = b1g @ Wk + bk
        bv_ = b1g @ Wv + bv
        bo_ = bv_ @ Wo + bo
        w1_ = g2[:, None] * W1
        b1_ = b2g @ W1 + b1
        wmap[f"wq_{l}"] = bfl(wq_)
        wmap[f"wk_{l}"] = bfl(wk_)
        wmap[f"wv_{l}"] = bfl(g1[:, None] * Wv)
        wmap[f"wo_{l}"] = bfl(Wo)
        wmap[f"pk_{l}"] = bfl(np.ascontiguousarray(pk_rev.T))
        wmap[f"pq_{l}"] = bfl(np.ascontiguousarray(pq_rev.T))
        w1t = bfl(w1_)
        wmap[f"w1a_{l}"] = np.ascontiguousarray(w1t[:, :, :FF // 2])
        wmap[f"w1b_{l}"] = np.ascontiguousarray(w1t[:, :, FF // 2:])
        w2t = bfl(W2)
        wmap[f"w2a_{l}"] = np.ascontiguousarray(w2t[:, :FTH, :])
        wmap[f"w2b_{l}"] = np.ascontiguousarray(w2t[:, FTH:, :])
        wmap[f"bq_{l}"] = _tile_bias(bq_)
        wmap[f"bk_{l}"] = _tile_bias(bk_)
        wmap[f"bo_{l}"] = _tile_bias(bo_)
        wmap[f"b1_{l}"] = _tile_bias(b1_)
        wmap[f"b2_{l}"] = _tile_bias(b2)
    return wmap


_NC_CACHE = {}


def _get_nc(n_layers=N_LAYERS):
    if n_layers not in _NC_CACHE:
        _NC_CACHE[n_layers] = build(n_layers)
    return _NC_CACHE[n_layers]


def run(inputs, n_layers=N_LAYERS, trace=False):
    nc = _get_nc(n_layers)
    wmap = prep_weights(inputs, n_layers)
    tok = np.asarray(inputs["token_ids"], np.int32)
    seg = np.asarray(inputs["segment_info"], np.int32)
    in_maps = []
    for b in range(B):
        m = dict(wmap)
        m["token_ids"] = np.ascontiguousarray(tok[b].reshape(S, 1))
        m["segment_info"] = np.ascontiguousarray(seg[b].reshape(S, 1))
        in_maps.append(m)
    res = run_bass_kernel_spmd(nc, in_maps, core_ids=list(range(N_CORES)), trace=trace)
    outs = []
    for b in range(B):
        o = res.results[b]["out"]                      # [128, CT, S]
        o = o.transpose(1, 0, 2).reshape(HID, S).T     # -> [S, HID]
        outs.append(o)
    out = np.stack(outs).astype(np.float32)
    return out, res


def kernel(**inputs):
    out, _ = run(inputs)
    return out


# revision 15
# speedup vs baseline: 1.1036x; 1.0375x over previous
"""DeBERTa-style 12-layer transformer forward on 8 Trainium2 NeuronCores.

Strategy: data-parallel over batch (B=8 -> 1 sequence per core). Each core runs
the full 12-layer model on its [512, 768] activation, kept TRANSPOSED in SBUF
([HID on partitions, S on free]) so every projection is a plain PE matmul with
naturally-laid-out weights. No collectives.

DeBERTa relative-position terms: since S == M == 512, rel(q,k) = q-k+512 never
clips.  With host-reversed positional projections (pk_rev[r'] = pk[1023-r']):
    c2p[q, k]  = qpk_rev[q, 511-q+k]   (same-partition diagonal slice)
    p2cT[k, q] = kpq_rev[k, 511-k+q]   (same-partition diagonal slice)
Both are per-partition shifted contiguous row reads, which a SBUF->SBUF DMA
expresses as a single 3-dim access pattern with partition stride (pitch-1).
c2p is then block-transposed on the PE into [k, q] orientation, and p2cT is
accumulated onto it with an accumulating SWDGE DMA.

Softmax: logits are small (inputs are LN'd, weights ~0.02), so exp without
max-subtraction is numerically safe. The key-padding mask multiplies V rows
and the appended ones-column of V~ = [V*m | m], whose matmul row 64 yields the
softmax denominator for free; division is applied to the [64, 512] per-head
context via a TensorE reciprocal broadcast.

LayerNorm gain/bias are folded into the following projection weights on the
host; attention scale 1/sqrt(3D) is folded into Wq and Wpq.
"""

import math
import os
import sys

import numpy as np

sys.path.insert(0, "/opt/trn_rl_repo")

import ml_dtypes  # noqa: F401

import concourse.bass as bass
import concourse.mybir as mybir
import concourse.tile as tile
from concourse import bacc
from concourse.bass_utils import run_bass_kernel_spmd
from concourse.masks import make_identity

dt = mybir.dt
AF = mybir.ActivationFunctionType
ALU = mybir.AluOpType

B, S, HID, L, NH, V, M = 8, 512, 768, 12, 12, 30522, 512
FF = 4 * HID
D = HID // NH           # 64
CT = HID // 128         # 6 c-tiles
ST = S // 128           # 4 s-tiles
FT = FF // 128          # 24 f-tiles
FTH = FT // 2           # 12 (FFN half)
R2 = 2 * M              # 1024
RT = R2 // 512          # 2 chunks of the r axis
BAND = 640              # needed qpk band width per 128-row tile
SCALE = 1.0 / math.sqrt(3.0 * D)
EPS = 1e-5

N_LAYERS = int(os.environ.get("KERNEL_N_LAYERS", str(L)))
ABLATE = set(x for x in os.environ.get("KERNEL_ABLATE", "").split(",") if x)
N_CORES = 8

bf16 = dt.bfloat16
f32 = dt.float32
f32r = dt.float32r


def _diag_ap(t, n_tiles, band, width, base):
    """AP reading t[p, i, base - p + k] for k in [0, width): per-partition
    backward-shifted contiguous rows, one DMA for all n_tiles sub-tiles."""
    src = t[:, :, :].copy()
    pitch = src.ap[0][0]
    out = src.copy()
    out.ap = src.ap[:0] + [[pitch - 1, 128], [band, n_tiles], [1, width]]
    out.offset = src.offset + base
    return out


def build(n_layers=N_LAYERS):
    nc = bacc.Bacc(None, target_bir_lowering=False, debug=False)
    with tile.TileContext(nc) as tc:
        with tc.tile_pool(name="dram", bufs=1, space="DRAM") as dram:
            tok_d = dram.tile([S, 1], dt.int32, kind="ExternalInput", name="token_ids", uniquify=False)
            seg_d = dram.tile([S, 1], dt.int32, kind="ExternalInput", name="segment_info", uniquify=False)
            emb_d = dram.tile([V, HID], f32, kind="ExternalInput", name="tok_emb", uniquify=False)
            sege_d = dram.tile([3, HID], f32, kind="ExternalInput", name="seg_emb", uniquify=False)
            pe_d = dram.tile([128, ST, HID], f32, kind="ExternalInput", name="pe", uniquify=False)
            lnfg_d = dram.tile([128, CT], f32, kind="ExternalInput", name="lnf_g", uniquify=False)
            lnfb_d = dram.tile([128, CT], f32, kind="ExternalInput", name="lnf_b", uniquify=False)
            out_d = dram.tile([128, CT, S], f32, kind="ExternalOutput", name="out", uniquify=False)

            WD = {}
            for l in range(n_layers):
                for nm, sh in [("wq", [128, CT, HID]), ("wk", [128, CT, HID]),
                               ("wv", [128, CT, HID]), ("wo", [128, CT, HID]),
                               ("pk", [128, CT, R2]), ("pq", [128, CT, R2]),
                               ("w1a", [128, CT, FF // 2]), ("w1b", [128, CT, FF // 2]),
                               ("w2a", [128, FTH, HID]), ("w2b", [128, FTH, HID])]:
                    WD[nm, l] = dram.tile(sh, bf16, kind="ExternalInput", name=f"{nm}_{l}", uniquify=False)
                for nm, sh in [("bq", [128, CT]), ("bk", [128, CT]), ("bo", [128, CT]),
                               ("b1", [128, FT]), ("b2", [128, CT])]:
                    WD[nm, l] = dram.tile(sh, f32, kind="ExternalInput", name=f"{nm}_{l}", uniquify=False)

            with nc.allow_low_precision("bf16/f32r kernel by design"):
                _build_body(nc, tc, n_layers, tok_d, seg_d, emb_d, sege_d, pe_d,
                            lnfg_d, lnfb_d, out_d, WD)

    nc.compile()
    return nc


def _build_body(nc, tc, n_layers, tok_d, seg_d, emb_d, sege_d, pe_d,
                lnfg_d, lnfb_d, out_d, WD):
    with tc.tile_pool(name="persist", bufs=1) as pers, \
         tc.tile_pool(name="whid", bufs=2) as whid, \
         tc.tile_pool(name="wff", bufs=2) as wff, \
         tc.tile_pool(name="wb", bufs=2) as wb, \
         tc.tile_pool(name="act", bufs=1) as act, \
         tc.tile_pool(name="scr", bufs=1) as scr, \
         tc.tile_pool(name="hscr", bufs=2) as hscr, \
         tc.tile_pool(name="bnd", bufs=2) as bnd, \
         tc.tile_pool(name="hscr2", bufs=2) as hscr2, \
         tc.tile_pool(name="ppA", bufs=4, space="PSUM") as ppA, \
         tc.tile_pool(name="ppB", bufs=2, space="PSUM") as ppB:

        # ---------------- constants ----------------
        ident_f = pers.tile([128, 128], f32, name="ident_f")
        make_identity(nc, ident_f[:, :])
        ident_b = pers.tile([128, 128], bf16, name="ident_b")
        nc.vector.tensor_copy(out=ident_b[:, :], in_=ident_f[:, :])
        ones_f = pers.tile([128, 1], f32, name="ones_f")
        nc.vector.memset(ones_f[:, :], 1.0)
        ones_col_r = pers.tile([128, 1], f32r, name="ones_col_r")
        nc.vector.tensor_copy(out=ones_col_r[:, :], in_=ones_f[:, :])
        ones_rf = pers.tile([1, 128], f32, name="ones_rf")
        nc.vector.memset(ones_rf[:, :], 1.0)
        ones_row_r = pers.tile([1, 128], f32r, name="ones_row_r")
        nc.vector.tensor_copy(out=ones_row_r[:, :], in_=ones_rf[:, :])
        eps_t = pers.tile([1, 1], f32, name="eps_t")
        nc.vector.memset(eps_t[:, :], EPS)

        # ---------------- embedding (DMA-only accumulate) ----------------
        tok_sb = pers.tile([128, ST], dt.int32, name="tok_sb")
        seg_sb = pers.tile([128, ST], dt.int32, name="seg_sb")
        for t in range(ST):
            nc.sync.dma_start(out=tok_sb[:, t:t + 1], in_=tok_d[128 * t:128 * (t + 1), :])
            nc.sync.dma_start(out=seg_sb[:, t:t + 1], in_=seg_d[128 * t:128 * (t + 1), :])

        m01_f = pers.tile([128, ST], f32, name="m01_f")
        m01_b = pers.tile([128, ST], bf16, name="m01_b")
        nc.vector.tensor_scalar(out=m01_f[:, :], in0=tok_sb[:, :], scalar1=0,
                                scalar2=None, op0=ALU.not_equal)
        nc.vector.tensor_copy(out=m01_b[:, :], in_=m01_f[:, :])

        xtmp = act.tile([128, ST, HID], f32, name="xtmp")
        x_nat = xtmp
        nc.sync.dma_start(out=x_nat[:, :, :], in_=pe_d[:, :, :])
        for t in range(ST):
            nc.gpsimd.indirect_dma_start(
                out=x_nat[:, t, :], out_offset=None, in_=emb_d[:, :],
                in_offset=bass.IndirectOffsetOnAxis(ap=tok_sb[:, t:t + 1], axis=0),
                compute_op=ALU.add)
            nc.gpsimd.indirect_dma_start(
                out=x_nat[:, t, :], out_offset=None, in_=sege_d[:, :],
                in_offset=bass.IndirectOffsetOnAxis(ap=seg_sb[:, t:t + 1], axis=0),
                compute_op=ALU.add)

        # transpose x_nat [S, HID] -> xT [HID, S] (f32r residual)
        xT = act.tile([128, CT, S], f32r, name="xT")
        for ct in range(CT):
            tp = ppA.tile([128, 512], f32, name="ps", tag="ps")
            for t in range(ST):
                nc.tensor.transpose(out=tp[:, 128 * t:128 * (t + 1)],
                                    in_=x_nat[:, t, 128 * ct:128 * (ct + 1)],
                                    identity=ident_f[:, :])
            nc.vector.tensor_copy(out=xT[:, ct, :], in_=tp[:, :])

        # ---------------- helper: layernorm ----------------
        def layernorm(x_in, h_out, g_ap=None, b_ap=None):
            if not callable(x_in):
                x_t = x_in; x_in = lambda kt: x_t[:, kt, :]
            if not callable(h_out):
                h_t = h_out; h_out = lambda ct: h_t[:, ct, :]
            sum_ps = ppA.tile([1, 512], f32, name="ln_sum", tag="ps")
            for kt in range(CT):
                nc.tensor.matmul(sum_ps[:, :], ones_col_r[:, :], x_in(kt),
                                 start=(kt == 0), stop=(kt == CT - 1))
            sq_ps = ppA.tile([1, 512], f32, name="ln_sq", tag="ps")
            for kt in range(CT):
                x2 = scr.tile([128, 512], f32r, name="ln_x2")
                nc.scalar.activation(out=x2[:, :], in_=x_in(kt), func=AF.Square)
                nc.tensor.matmul(sq_ps[:, :], ones_col_r[:, :], x2[:, :],
                                 start=(kt == 0), stop=(kt == CT - 1))
            mu = scr.tile([1, 512], f32, name="ln_mu")
            nc.vector.tensor_scalar(out=mu[:, :], in0=sum_ps[:, :], scalar1=1.0 / HID,
                                    scalar2=None, op0=ALU.mult)
            msq = scr.tile([1, 512], f32, name="ln_msq")
            nc.vector.tensor_tensor(out=msq[:, :], in0=mu[:, :], in1=mu[:, :], op=ALU.mult)
            var = scr.tile([1, 512], f32, name="ln_var")
            nc.vector.scalar_tensor_tensor(out=var[:, :], in0=sq_ps[:, :], scalar=1.0 / HID,
                                           in1=msq[:, :], op0=ALU.mult, op1=ALU.subtract)
            sig = scr.tile([1, 512], f32, name="ln_sig")
            nc.scalar.activation(out=sig[:, :], in_=var[:, :], func=AF.Sqrt, bias=eps_t[:, :])
            stack = scr.tile([1, R2], f32r, name="ln_stack")
            nc.vector.reciprocal(out=stack[:, 0:512], in_=sig[:, :])
            nc.vector.tensor_tensor(out=stack[:, 512:R2], in0=mu[:, :],
                                    in1=stack[:, 0:512], op=ALU.mult)
            maps = scr.tile([128, R2], f32r, name="ln_maps")
            for c in range(2):
                mp = ppA.tile([128, 512], f32, name="ln_mp", tag="ps")
                nc.tensor.matmul(mp[:, :], ones_row_r[:1, :], stack[:, 512 * c:512 * (c + 1)],
                                 start=True, stop=True)
                nc.scalar.activation(out=maps[:, 512 * c:512 * (c + 1)], in_=mp[:, :], func=AF.Copy)
            for ct in range(CT):
                t1 = scr.tile([128, 512], f32, name="ln_t1")
                nc.vector.tensor_tensor(out=t1[:, :], in0=x_in(ct),
                                        in1=maps[:, 0:512], op=ALU.mult)
                if g_ap is None:
                    nc.vector.tensor_tensor(out=h_out(ct), in0=t1[:, :],
                                            in1=maps[:, 512:R2], op=ALU.subtract)
                else:
                    t2 = scr.tile([128, 512], f32, name="ln_t2")
                    nc.vector.tensor_tensor(out=t2[:, :], in0=t1[:, :],
                                            in1=maps[:, 512:R2], op=ALU.subtract)
                    nc.scalar.activation(out=h_out(ct), in_=t2[:, :], func=AF.Identity,
                                         scale=g_ap[:, ct:ct + 1], bias=b_ap[:, ct:ct + 1])

        hT = act.tile([128, CT, S], bf16, name="hT")
        qT = act.tile([128, CT, S], bf16, name="qT")
        kT = act.tile([128, CT, S], bf16, name="kT")
        vS = act.tile([128, ST, NH, D + 1], bf16, name="vS")
        pkT = act.tile([128, CT, R2], bf16, name="pkT")
        pqT = act.tile([128, CT, R2], bf16, name="pqT")
        ctxT = act.tile([128, CT, S], bf16, name="ctxT")
        m1q = act.tile([128, CT, S], bf16, name="m1q")

        # ---------------- layers ----------------
        reps = int(os.environ.get("KERNEL_REPS", "1"))
        for l in [x % n_layers for x in range(n_layers * reps)]:
            # ---- LN1 -> hT (bf16)
            layernorm(xT, hT)

            # ---- q/k projections (transposed out) with folded bias
            for wi, (wnm, bnm, o_t) in enumerate([("wq", "bq", qT), ("wk", "bk", kT)]):
                w_t = whid.tile([128, CT, HID], bf16, name=wnm, tag="whid")
                eng = nc.sync if wi == 0 else nc.scalar
                eng.dma_start(out=w_t[:, :, :], in_=WD[wnm, l][:, :, :])
                b_t = wb.tile([128, CT], f32, name=bnm, tag="wbias")
                nc.sync.dma_start(out=b_t[:, :], in_=WD[bnm, l][:, :])
                for ot in range(CT):
                    pp = ppA.tile([128, 512], f32, name="proj_ps", tag="ps")
                    for kt in range(CT):
                        nc.tensor.matmul(pp[:, :], w_t[:, kt, 128 * ot:128 * (ot + 1)],
                                         hT[:, kt, :], start=(kt == 0), stop=(kt == CT - 1))
                    nc.scalar.activation(out=o_t[:, ot, :], in_=pp[:, :], func=AF.Identity,
                                         bias=b_t[:, ot:ot + 1])

            # ---- v natural [s, h, d] with mask folded; ones column = m01
            wv = whid.tile([128, CT, HID], bf16, name="wv", tag="whid")
            nc.scalar.dma_start(out=wv[:, :, :], in_=WD["wv", l][:, :, :])
            for st in range(ST):
                for c in range(2):  # two 384-wide chunks of d
                    pp = ppA.tile([128, 512], f32, name="v_ps", tag="ps")
                    for kt in range(CT):
                        nc.tensor.matmul(pp[:, 0:384], hT[:, kt, 128 * st:128 * (st + 1)],
                                         wv[:, kt, 384 * c:384 * (c + 1)],
                                         start=(kt == 0), stop=(kt == CT - 1))
                    src = pp[:, 0:384].rearrange("p (h d) -> p h d", d=D)
                    nc.scalar.activation(out=vS[:, st, 6 * c:6 * (c + 1), 0:D], in_=src,
                                         func=AF.Copy, scale=m01_f[:, st:st + 1])
                icol = m01_b[:, st:st + 1].copy()
                icol.ap = icol.ap[:0] + [[icol.ap[0][0], 128], [0, NH], [1, 1]]
                nc.vector.tensor_copy(out=vS[:, st, :, D:D + 1], in_=icol)

            # ---- positional projections precomputed on host; just load
            nc.sync.dma_start(out=pkT[:, :, :], in_=WD["pk", l][:, :, :])
            nc.scalar.dma_start(out=pqT[:, :, :], in_=WD["pq", l][:, :, :])

            # ---- attention per head
            for h in range(NH if "attn" not in ABLATE else 0):
                hp = 64 * (h % 2)
                hc = h // 2
                q_h = qT[hp:hp + 64, hc, :]
                k_h = kT[hp:hp + 64, hc, :]

                bq_t = bnd.tile([128, ST, BAND], bf16, name="band_q")
                bk_t = bnd.tile([128, ST, BAND], bf16, name="band_k")
                for src_T, posT, bt in ([(qT, pkT, bq_t), (kT, pqT, bk_t)] if "bands" not in ABLATE else []):
                    s_h = src_T[hp:hp + 64, hc, :]
                    p_h = posT[hp:hp + 64, hc, :]
                    for t in range(ST):
                        bs = 384 - 128 * t
                        bp = ppB.tile([128, BAND], f32, name="band_ps", tag="band")
                        nc.tensor.matmul(bp[:, 0:512], s_h[:, 128 * t:128 * (t + 1)],
                                         p_h[:, bs:bs + 512], start=True, stop=True)
                        nc.tensor.matmul(bp[:, 512:BAND], s_h[:, 128 * t:128 * (t + 1)],
                                         p_h[:, bs + 512:bs + BAND], start=True, stop=True)
                        if t % 2 == 0:
                            nc.scalar.activation(out=bt[:, t, :], in_=bp[:, :], func=AF.Copy)
                        else:
                            nc.vector.tensor_copy(out=bt[:, t, :], in_=bp[:, :])

                # c2p [q, k] and p2cT [k, q] via diagonal skew DMAs, one per
                # HWDGE ring so they run concurrently
                c2p = hscr.tile([128, ST, 512], bf16, name="c2p")
                p2cT = hscr.tile([128, ST, 512], bf16, name="p2cT")
                if "skew" not in ABLATE:
                    e1, e2 = (nc.sync, nc.scalar) if h % 2 == 0 else (nc.scalar, nc.sync)
                    e1.dma_start(out=c2p[:, :, :], in_=_diag_ap(bq_t, ST, BAND, 512, 127))
                    e2.dma_start(out=p2cT[:, :, :], in_=_diag_ap(bk_t, ST, BAND, 512, 127))
                # pos[k,q] = transpose(c2p) + p2cT, fused into the PSUM eviction
                pos = hscr.tile([128, ST, 512], bf16, name="pos")
                for u in range(ST):
                    tp = ppA.tile([128, 512], bf16, name="c2pT_ps", tag="ps")
                    if "transpose" not in ABLATE:
                        for t in range(ST):
                            nc.tensor.transpose(out=tp[:, 128 * t:128 * (t + 1)],
                                                in_=c2p[:, t, 128 * u:128 * (u + 1)],
                                                identity=ident_b[:, :])
                    if "posadd" not in ABLATE:
                        nc.vector.tensor_tensor(out=pos[:, u, :], in0=tp[:, :],
                                                in1=p2cT[:, u, :], op=ALU.add)

                # scoresT per k-tile: c2c (PSUM) + pos -> exp (bf16) -> AV
                av = ppA.tile([128, 512], f32, name="av_ps", tag="ps")
                for u in range(ST):
                    cp = ppA.tile([128, 512], f32, name="c2c_ps", tag="ps")
                    nc.tensor.matmul(cp[:, :], k_h[:, 128 * u:128 * (u + 1)], q_h[:, :],
                                     start=True, stop=True)
                    sf = hscr2.tile([128, 512], f32, name="s_f32")
                    ex = hscr2.tile([128, 512], bf16, name="expT")
                    if "exp" not in ABLATE:
                        nc.vector.tensor_tensor(out=sf[:, :], in0=cp[:, :], in1=pos[:, u, :], op=ALU.add)
                        nc.scalar.activation(out=ex[:, :], in_=sf[:, :], func=AF.Exp)
                    nc.tensor.matmul(av[0:D + 1, :], vS[:, u, h, :], ex[:, :],
                                     start=(u == 0), stop=(u == ST - 1))

                rec = hscr2.tile([1, 512], f32r, name="rec")
                nc.vector.reciprocal(out=rec[:, :], in_=av[D:D + 1, :])
                rm = ppA.tile([128, 512], f32, name="rmap_ps", tag="ps")
                nc.tensor.matmul(rm[0:D, :], ones_row_r[:1, 0:D], rec[:, :], start=True, stop=True)
                rms = hscr2.tile([64, 512], f32r, name="rmap_sb")
                nc.scalar.activation(out=rms[:, :], in_=rm[0:D, :], func=AF.Copy)
                nc.vector.tensor_tensor(out=ctxT[hp:hp + 64, hc, :], in0=av[0:D, :],
                                        in1=rms[:, :], op=ALU.mult)

            # ---- attention out proj + residual
            wo = whid.tile([128, CT, HID], bf16, name="wo", tag="whid")
            nc.sync.dma_start(out=wo[:, :, :], in_=WD["wo", l][:, :, :])
            bo = wb.tile([128, CT], f32, name="bo", tag="wbias")
            nc.sync.dma_start(out=bo[:, :], in_=WD["bo", l][:, :])
            for ot in range(CT):
                pp = ppA.tile([128, 512], f32, name="wo_ps", tag="ps")
                for kt in range(CT):
                    nc.tensor.matmul(pp[:, :], wo[:, kt, 128 * ot:128 * (ot + 1)],
                                     ctxT[:, kt, :], start=(kt == 0), stop=(kt == CT - 1))
                nc.vector.scalar_tensor_tensor(out=xT[:, ot, :], in0=pp[:, :],
                                               scalar=bo[:, ot:ot + 1], in1=xT[:, ot, :],
                                               op0=ALU.add, op1=ALU.add)

            # ---- LN2 -> h2 (reuse hT)
            layernorm(xT, hT)

            # ---- FFN in quarters (m1 buffer reused; xT updated per quarter)
            b1 = wb.tile([128, FT], f32, name="b1", tag="wb1")
            nc.sync.dma_start(out=b1[:, :], in_=WD["b1", l][:, :])
            b2 = wb.tile([128, CT], f32, name="b2", tag="wbias")
            nc.sync.dma_start(out=b2[:, :], in_=WD["b2", l][:, :])
            for qtr in range(4 if "ffn" not in ABLATE else 0):
                half, qh = divmod(qtr, 2)
                w1nm = "w1a" if half == 0 else "w1b"
                w2nm = "w2a" if half == 0 else "w2b"
                w1 = wff.tile([128, CT, 768], bf16, name="w1", tag="wff")
                nc.scalar.dma_start(out=w1[:, :, :], in_=WD[w1nm, l][:, :, 768 * qh:768 * (qh + 1)])
                for ft in range(CT):
                    pp = ppA.tile([128, 512], f32, name="ff1_ps", tag="ps")
                    for kt in range(CT):
                        nc.tensor.matmul(pp[:, :], w1[:, kt, 128 * ft:128 * (ft + 1)],
                                         hT[:, kt, :], start=(kt == 0), stop=(kt == CT - 1))
                    fg = CT * qtr + ft
                    nc.scalar.activation(out=m1q[:, ft, :], in_=pp[:, :], func=AF.Gelu,
                                         bias=b1[:, fg:fg + 1])
                w2 = wff.tile([128, CT, HID], bf16, name="w2", tag="wff")
                nc.sync.dma_start(out=w2[:, :, :], in_=WD[w2nm, l][:, 6 * qh:6 * (qh + 1), :])
                for ot in range(CT):
                    pp = ppA.tile([128, 512], f32, name="ff2_ps", tag="ps")
                    for kt in range(CT):
                        nc.tensor.matmul(pp[:, :], w2[:, kt, 128 * ot:128 * (ot + 1)],
                                         m1q[:, kt, :], start=(kt == 0), stop=(kt == CT - 1))
                    if qtr < 3:
                        nc.vector.tensor_tensor(out=xT[:, ot, :], in0=pp[:, :],
                                                in1=xT[:, ot, :], op=ALU.add)
                    else:
                        nc.vector.scalar_tensor_tensor(out=xT[:, ot, :], in0=pp[:, :],
                                                       scalar=b2[:, ot:ot + 1], in1=xT[:, ot, :],
                                                       op0=ALU.add, op1=ALU.add)

        # ---------------- final LN + output ----------------
        lnfg = pers.tile([128, CT], f32, name="lnfg")
        lnfb = pers.tile([128, CT], f32, name="lnfb")
        nc.sync.dma_start(out=lnfg[:, :], in_=lnfg_d[:, :])
        nc.sync.dma_start(out=lnfb[:, :], in_=lnfb_d[:, :])
        xf_base = xtmp[:, :, :].copy()
        pitch = xf_base.ap[0][0]
        def xf_view(ct):
            v = xf_base.copy()
            v.ap = xf_base.ap[:0] + [[pitch, 128], [1, 512]]
            v.offset = xf_base.offset + 512 * ct
            return v
        layernorm(xT, xf_view, g_ap=lnfg, b_ap=lnfb)
        xf_all = xf_base.copy()
        xf_all.ap = xf_base.ap[:0] + [[pitch, 128], [512, CT], [1, 512]]
        nc.sync.dma_start(out=out_d[:, :, :], in_=xf_all)


# ---------------------------------------------------------------------------
# host side
# ---------------------------------------------------------------------------

def _tile_kxm(w):
    """[K, N] -> [128, K//128, N] partition-tiled."""
    K, N = w.shape
    return np.ascontiguousarray(w.reshape(K // 128, 128, N).transpose(1, 0, 2))


def _tile_bias(b):
    n = b.shape[0]
    return np.ascontiguousarray(b.reshape(n // 128, 128).T)


def _sinusoidal_pe(seq_len, dim):
    pos = np.arange(seq_len, dtype=np.float32)[:, None]
    i = np.arange(dim // 2, dtype=np.float32)[None, :]
    ang = pos / np.power(10000.0, 2.0 * i / dim)
    pe = np.zeros((seq_len, dim), np.float32)
    pe[:, 0::2] = np.sin(ang)
    pe[:, 1::2] = np.cos(ang)
    return pe


def prep_weights(inputs, n_layers=N_LAYERS):
    f = lambda k: np.asarray(inputs[k], np.float32)
    wmap = {}
    wmap["tok_emb"] = f("tok_emb")
    wmap["seg_emb"] = f("seg_emb")
    pe = _sinusoidal_pe(S, HID)
    wmap["pe"] = np.ascontiguousarray(pe.reshape(ST, 128, HID).transpose(1, 0, 2))
    wmap["lnf_g"] = _tile_bias(f("lnf_g"))
    wmap["lnf_b"] = _tile_bias(f("lnf_b"))
    bfl = lambda x: _tile_kxm(x).astype(ml_dtypes.bfloat16)
    for l in range(n_layers):
        g1, b1g = f("ln1_g")[l], f("ln1_b")[l]
        g2, b2g = f("ln2_g")[l], f("ln2_b")[l]
        Wq, Wk, Wv, Wo = f("Wq")[l], f("Wk")[l], f("Wv")[l], f("Wo")[l]
        Wpk, Wpq = f("Wpk")[l], f("Wpq")[l]
        rel = f("rel_emb")[l]
        W1, W2 = f("W1")[l], f("W2")[l]
        bq, bk, bv, bo = f("bq")[l], f("bk")[l], f("bv")[l], f("bo")[l]
        b1, b2 = f("b1")[l], f("b2")[l]

        pk_rev = (rel[::-1] @ Wpk)            # [R2, HID] reversed r
        pq_rev = (rel[::-1] @ (Wpq * SCALE))
        wq_ = (g1[:, None] * Wq) * SCALE
        bq_ = (b1g @ Wq + bq) * SCALE
        wk_ = g1[:, None] * Wk
        bk_ = b1g @ Wk + bk
        bv_ = b1g @ Wv + bv
        bo_ = bv_ @ Wo + bo
        w1_ = g2[:, None] * W1
        b1_ = b2g @ W1 + b1
        wmap[f"wq_{l}"] = bfl(wq_)
        wmap[f"wk_{l}"] = bfl(wk_)
        wmap[f"wv_{l}"] = bfl(g1[:, None] * Wv)
        wmap[f"wo_{l}"] = bfl(Wo)
        wmap[f"pk_{l}"] = bfl(np.ascontiguousarray(pk_rev.T))
        wmap[f"pq_{l}"] = bfl(np.ascontiguousarray(pq_rev.T))
        w1t = bfl(w1_)
        wmap[f"w1a_{l}"] = np.ascontiguousarray(w1t[:, :, :FF // 2])
        wmap[f"w1b_{l}"] = np.ascontiguousarray(w1t[:, :, FF // 2:])
        w2t = bfl(W2)
        wmap[f"w2a_{l}"] = np.ascontiguousarray(w2t[:, :FTH, :])
        wmap[f"w2b_{l}"] = np.ascontiguousarray(w2t[:, FTH:, :])
        wmap[f"bq_{l}"] = _tile_bias(bq_)
        wmap[f"bk_{l}"] = _tile_bias(bk_)
        wmap[f"bo_{l}"] = _tile_bias(bo_)
        wmap[f"b1_{l}"] = _tile_bias(b1_)
        wmap[f"b2_{l}"] = _tile_bias(b2)
    return wmap


_NC_CACHE = {}


def _get_nc(n_layers=N_LAYERS):
    if n_layers not in _NC_CACHE:
        _NC_CACHE[n_layers] = build(n_layers)
    return _NC_CACHE[n_layers]


def run(inputs, n_layers=N_LAYERS, trace=False):
    nc = _get_nc(n_layers)
    wmap = prep_weights(inputs, n_layers)
    tok = np.asarray(inputs["token_ids"], np.int32)
    seg = np.asarray(inputs["segment_info"], np.int32)
    in_maps = []
    for b in range(B):
        m = dict(wmap)
        m["token_ids"] = np.ascontiguousarray(tok[b].reshape(S, 1))
        m["segment_info"] = np.ascontiguousarray(seg[b].reshape(S, 1))
        in_maps.append(m)
    res = run_bass_kernel_spmd(nc, in_maps, core_ids=list(range(N_CORES)), trace=trace)
    outs = []
    for b in range(B):
        o = res.results[b]["out"]                      # [128, CT, S]
        o = o.transpose(1, 0, 2).reshape(HID, S).T     # -> [S, HID]
        outs.append(o)
    out = np.stack(outs).astype(np.float32)
    return out, res


def kernel(**inputs):
    out, _ = run(inputs)
    return out
